# revision 2
# baseline (speedup 1.0000x reference)
"""CRLLoss (majority-class masked mean cross-entropy) on 8 trn2 NeuronCores.

Math: loss = sum_i keep_i * (logsumexp(x_i) - x_i[label_i]) / max(sum_i keep_i, 1)
where keep_i = label_i not in min_classes.

Sharding (data-parallel over N): each core gets 32768 rows of cls_score.
On-core layout is partition-major: partition p owns rows p*256..p*256+255 of
its shard, so every DMA chunk is a contiguous 32KB-per-partition block.

Per chunk of 8 rows/partition:
  - ScalarE: exp-with-accumulate -> per-row sum(exp(x))  (no max subtraction
    needed: |x| ~ N(0,1), exp stays in fp32 range)
  - VectorE: scalar_tensor_tensor (iota == label) * x with free-dim accum ->
    per-row x[label] gather
Epilogue reduces log(sumexp) - gather against the keep mask to two scalars per
core; host sums the 8 partial pairs and divides.
"""

import numpy as np

import concourse.bass as bass
import concourse.tile as tile
from concourse import bacc, mybir
from concourse.bass_utils import run_bass_kernel_spmd

LOSS_WEIGHT = 1.0

N, C = 262144, 1000
NCORES = 8
ROWS = N // NCORES          # 32768 rows per core
P = 128                     # SBUF partitions
RPP = ROWS // P             # 256 rows per partition
KCH = 8                     # rows per partition per DMA chunk (32KB/partition)
NCH = RPP // KCH            # 32 chunks

_F32 = mybir.dt.float32

_cached_nc = None


def _build_nc():
    nc = bacc.Bacc("TRN2", debug=False, target_bir_lowering=False)

    x = nc.dram_tensor("x", [ROWS, C], _F32, kind="ExternalInput")
    labf = nc.dram_tensor("labf", [P, RPP], _F32, kind="ExternalInput")
    keepf = nc.dram_tensor("keepf", [P, RPP], _F32, kind="ExternalInput")
    iotaf = nc.dram_tensor("iotaf", [P, C], _F32, kind="ExternalInput")
    out = nc.dram_tensor("out", [1, 2], _F32, kind="ExternalOutput")

    # partition-major view: [p, r, c] -> row p*RPP + r of the shard
    xr = x.ap().rearrange("(p r) c -> p r c", p=P)

    with tile.TileContext(nc) as tc:
        with (
            tc.tile_pool(name="xchunks", bufs=3) as xp,
            tc.tile_pool(name="consts", bufs=1) as consts,
            tc.tile_pool(name="scratch", bufs=1) as scr,
            tc.tile_pool(name="psum", bufs=1, space="PSUM") as psp,
        ):
            iota_s = consts.tile([P, C], _F32)
            nc.sync.dma_start(iota_s[:], iotaf.ap())
            labf_s = consts.tile([P, RPP], _F32)
            nc.sync.dma_start(labf_s[:], labf.ap())
            keepf_s = consts.tile([P, RPP], _F32)
            nc.sync.dma_start(keepf_s[:], keepf.ap())

            sumexp = consts.tile([P, RPP], _F32)
            gath = consts.tile([P, RPP], _F32)

            scr_act = scr.tile([P, C], _F32, tag="scr_act")
            scr_dve = scr.tile([P, C], _F32, tag="scr_dve")

            for ch in range(NCH):
                xt = xp.tile([P, KCH, C], _F32)
                nc.sync.dma_start(xt[:], xr[:, ch * KCH:(ch + 1) * KCH, :])
                for k in range(KCH):
                    j = ch * KCH + k
                    nc.scalar.activation(
                        scr_act[:],
                        xt[:, k, :],
                        mybir.ActivationFunctionType.Exp,
                        accum_out=sumexp[:, j:j + 1],
                    )
                    nc.vector.scalar_tensor_tensor(
                        scr_dve[:],
                        iota_s[:],
                        labf_s[:, j:j + 1],
                        xt[:, k, :],
                        op0=mybir.AluOpType.is_equal,
                        op1=mybir.AluOpType.mult,
                        accum_out=gath[:, j:j + 1],
                    )

            # epilogue: two partial scalars
            logz = consts.tile([P, RPP], _F32)
            nc.scalar.activation(
                logz[:], sumexp[:], mybir.ActivationFunctionType.Ln
            )
            diff = consts.tile([P, RPP], _F32)
            nc.vector.tensor_tensor(
                diff[:], logz[:], gath[:], op=mybir.AluOpType.subtract
            )
            part = consts.tile([P, 2], _F32)
            ce_keep = consts.tile([P, RPP], _F32)
            nc.vector.tensor_tensor(
                ce_keep[:], diff[:], keepf_s[:], op=mybir.AluOpType.mult
            )
            nc.vector.tensor_reduce(
                part[:, 0:1],
                ce_keep[:],
                axis=mybir.AxisListType.X,
                op=mybir.AluOpType.add,
            )
            nc.vector.tensor_reduce(
                part[:, 1:2],
                keepf_s[:],
                axis=mybir.AxisListType.X,
                op=mybir.AluOpType.add,
            )
            ones = consts.tile([P, 1], _F32)
            nc.vector.memset(ones[:], 1.0)
            acc = psp.tile([1, 2], _F32)
            nc.tensor.matmul(acc[:], ones[:], part[:], start=True, stop=True)
            res = consts.tile([1, 2], _F32)
            nc.vector.tensor_copy(res[:], acc[:])
            nc.sync.dma_start(out.ap(), res[:])

    nc.compile()
    return nc


def kernel(cls_score, label, min_classes):
    global _cached_nc
    cls_score = np.ascontiguousarray(np.asarray(cls_score, dtype=np.float32))
    label = np.asarray(label)
    min_classes = np.asarray(min_classes)

    keep = (~np.isin(label, min_classes)).astype(np.float32)   # [N]
    labf = label.astype(np.float32)                            # [N]
    iotaf = np.broadcast_to(
        np.arange(C, dtype=np.float32), (P, C)
    ).copy()

    if _cached_nc is None:
        _cached_nc = _build_nc()
    nc = _cached_nc

    in_maps = []
    for s in range(NCORES):
        lo, hi = s * ROWS, (s + 1) * ROWS
        in_maps.append({
            "x": cls_score[lo:hi],
            "labf": labf[lo:hi].reshape(P, RPP),
            "keepf": keep[lo:hi].reshape(P, RPP),
            "iotaf": iotaf,
        })

    results = run_bass_kernel_spmd(nc, in_maps, core_ids=list(range(NCORES)))
    partials = np.stack([r["out"].reshape(2) for r in results.results])  # [8, 2]
    ce_sum = float(partials[:, 0].sum())
    keep_sum = float(partials[:, 1].sum())
    return np.array(LOSS_WEIGHT * ce_sum / max(keep_sum, 1.0), dtype=np.float32)
